# revision 9
# baseline (speedup 1.0000x reference)
"""TRN2 Bass kernel for nn_AttentionMatcher: 8-way row-sharded dense attention.

reference semantics (training branch, iseval=0):
    mt = N @ M.T; mt[diag] = 0
    attn = softmax(mt, axis=-1)
    out_attn = attn @ M
    gate = sigmoid(out_attn @ Wg.T + bg + gate_b)
    boosted = out_attn * gate + N * (1 - gate)
    return boosted[:, None, None, :]

Distribution: shard rows of N (1024/core on 8 cores), replicate M.

Per-core algorithm:
  - scoresT[m, n_loc] = MT.T-block @ NT in fp32r (full TensorE rate at
    free>=256; scores kept TRANSPOSED so no on-chip transposes anywhere)
  - expT = exp(scoresT - SHIFT) on ScalarE, fused PSUM->SBUF, output bf16.
    No per-row max is needed: scores ~ N(0, 16^2), so a constant shift keeps
    exp() finite (bf16 shares f32's exponent range) and softmax is
    shift-invariant.
  - out_attn_unnorm[n, 0:258] += expT-block.T @ MA-block in bf16 (FWL weight
    loads, half the HBM traffic of f32), where MA = [M | 1 | M@Wg.T]:
    column 256 accumulates the softmax denominator Z and column 257 the
    gate dot product U.Wg -- both for free inside mm2.
  - diagonal removal (SPMD-uniform): subtract the diagonal term
    exp(dot(N_i,M_i)-SHIFT) * MA_f32[i] per row using the per-core tensor
    MD = [M | 1 | M@Wg.T][rows of this shard] in f32.  (The reference sets
    the diag *score* to 0, i.e. weight exp(-max) ~ 1e-30 relative.)
  - epilogue (batched across the 4 n-tiles of a half): rz = 1/Z,
    gate = 1/(1+exp(-(U.Wg * rz + b))), out = gate*(U*rz - N) + N.

DMA layouts: MA/NF/MD/out are pre-transposed on the host to
[128 partitions, tiles, cols] so every partition's data is contiguous
(128 descriptors per chunk instead of 1024 -- the f32 row-interleaved
layout spent ~6us of Sync-queue issue time per MA chunk).
"""

import numpy as np

N_ROWS = 8192
EMBED = 256
NCORES = 8
NLOC = N_ROWS // NCORES  # 1024
NT_TILES = NLOC // 128   # 8 n-tiles per core
MT_TILES = N_ROWS // 128  # 64 m-tiles
SHIFT = 44.0

_cache: dict = {}


def _build_nc(repeat=1, loop_scope="all", ablate=""):
    import contextlib
    import concourse.bacc as bacc
    import concourse.mybir as mybir
    import concourse.tile as tile

    f32 = mybir.dt.float32
    f32r = mybir.dt.float32r
    bf16 = mybir.dt.bfloat16
    Exp = mybir.ActivationFunctionType.Exp
    mult = mybir.AluOpType.mult
    add = mybir.AluOpType.add

    nc = bacc.Bacc("TRN2", target_bir_lowering=False, debug=False,
                   num_devices=NCORES)

    d_MT = nc.dram_tensor("MT", (EMBED, N_ROWS), f32r, kind="ExternalInput")
    # MA pre-transposed host-side: [128, MT_TILES, 258] bf16, where
    # MA[p, b, :] = [M | 1 | M@Wg.T][b*128 + p, :]
    d_MA = nc.dram_tensor("MA", (128, MT_TILES, EMBED + 2), bf16,
                          kind="ExternalInput")
    d_NT = nc.dram_tensor("NT", (EMBED, NLOC), f32r, kind="ExternalInput")
    # NF/MD/out pre-transposed the same way: [128, NT_TILES, cols]
    d_NF = nc.dram_tensor("NF", (128, NT_TILES, EMBED), f32,
                          kind="ExternalInput")
    d_MD = nc.dram_tensor("MD", (128, NT_TILES, EMBED + 2), f32,
                          kind="ExternalInput")
    d_GB = nc.dram_tensor("GB", (128, 1), f32, kind="ExternalInput")
    d_out = nc.dram_tensor("out", (128, NT_TILES, EMBED), f32,
                           kind="ExternalOutput")

    K = 8  # m-chunks for DMA
    TPC = MT_TILES // K  # m-tiles per chunk

    with tile.TileContext(nc) as tc:
        with (
            tc.tile_pool(name="big", bufs=1) as big,
            tc.tile_pool(name="work", bufs=2) as work,
            tc.tile_pool(name="ps_s", bufs=2, space="PSUM") as ps_s,
            tc.tile_pool(name="ps_a", bufs=4, space="PSUM") as ps_a,
            (tc.For_i(0, repeat, 1) if repeat > 1 and loop_scope == "all"
             else contextlib.nullcontext()),
        ):
            # ---- resident inputs, DMA'd in consumption order ----
            # pass 1 needs only NT[:, 0:512]; split so compute starts early
            nt_ap = d_NT.ap().rearrange("(e p) n -> p e n", p=128)
            nt_sb = big.tile([128, 2, NLOC], f32r, tag="nt")
            nc.sync.dma_start(nt_sb[:, :, 0:512], nt_ap[:, :, 0:512])

            eb = big.tile([128, 1], f32, tag="eb")
            nc.gpsimd.memset(eb[:], -SHIFT)

            # warm the PE HAM clock-gate during the initial DMA wait with
            # dummy matmuls on zeroed tiles (~3.5us to reach max pstate)
            wz = big.tile([128, 128], f32r, tag="wz")
            nc.vector.memset(wz[:].bitcast(f32), 0.0)
            wzm = big.tile([128, 512], f32r, tag="wzm")
            nc.vector.memset(wzm[:].bitcast(f32), 0.0)
            out_sb = big.tile([128, NT_TILES, EMBED], f32, tag="outsb")
            wps = ps_s.tile([128, 1024], f32, tag="scores", name="warm_ps")
            for _ in range(7):
                nc.tensor.matmul(wps[:, 0:512], wz[:], wzm[:],
                                 start=True, stop=True)
            # keeper: dead-store into out_sb (fully overwritten by epilogue)
            nc.vector.tensor_copy(out_sb[:, 0, 0:4], wps[:, 0:4])

            # M forms, DMA'd in K chunks so compute can start early
            mt_ap = d_MT.ap().rearrange("(e p) m -> p e m", p=128)
            ma_ap = d_MA.ap()
            mt_ch = []
            ma_ch = []

            def _dma_chunk(k):
                mt_k = big.tile([128, 2, N_ROWS // K], f32r, tag=f"mt{k}",
                                name=f"mt{k}")
                CW = N_ROWS // K
                if k == 0:
                    # split the first chunk so the very first matmul can
                    # start after ~0.5MB instead of ~1MB of DMA
                    nc.sync.dma_start(
                        mt_k[:, :, 0:CW // 2], mt_ap[:, :, 0:CW // 2])
                    nc.sync.dma_start(
                        mt_k[:, :, CW // 2:CW], mt_ap[:, :, CW // 2:CW])
                else:
                    nc.sync.dma_start(
                        mt_k[:], mt_ap[:, :, k * CW:(k + 1) * CW])
                mt_ch.append(mt_k)
                ma_k = big.tile([128, TPC, EMBED + 2], bf16,
                                tag=f"ma{k}", name=f"ma{k}")
                nc.sync.dma_start(
                    ma_k[:], ma_ap[:, k * TPC:(k + 1) * TPC, :])
                ma_ch.append(ma_k)

            for k in range(K // 2):
                _dma_chunk(k)
            # second NT half mid-stream (needed at pass-2 start, ~0.5MB)
            nc.sync.dma_start(nt_sb[:, :, 512:NLOC], nt_ap[:, :, 512:NLOC])
            for k in range(K // 2, K):
                _dma_chunk(k)
            # epilogue-only data LAST: needed no earlier than the pass-1
            # epilogue (~60us); interleaving it mid-stream stalled chunks 4-7
            nf_sb = big.tile([128, NT_TILES, EMBED], f32, tag="nf")
            nc.sync.dma_start(nf_sb[:], d_NF.ap())
            md_sb = big.tile([128, NT_TILES, EMBED + 2], f32, tag="md")
            nc.sync.dma_start(md_sb[:], d_MD.ap())
            # GB holds -(bg + gate_b): used as exp(-(gd + b)) = exp(-gd + GB)
            gbn = big.tile([128, 1], f32, tag="gbn")
            nc.sync.dma_start(gbn[:], d_GB.ap())

            out_ap = d_out.ap()

            # diag correction weights, hoisted off the critical tail: runs on
            # otherwise-idle DVE/ACT once nf/md arrive (mid pass 1)
            negw_all = big.tile([128, NT_TILES], f32, tag="negw_all")
            for g in range(NT_TILES):
                tmp = work.tile([128, EMBED], f32, tag="tmp")
                diag = work.tile([128, 1], f32, tag="diag")
                nc.vector.tensor_mul(tmp[:], nf_sb[:, g, :], md_sb[:, g, 0:EMBED])
                nc.vector.reduce_sum(diag[:], tmp[:], axis=mybir.AxisListType.X)
                w = work.tile([128, 1], f32, tag="w")
                nc.scalar.activation(w[:], diag[:], Exp, bias=eb[:], scale=1.0)
                nc.vector.tensor_scalar_mul(negw_all[:, g:g + 1], w[:], -1.0)

            NPAIR = MT_TILES // 2

            def _mm2(pair, pexp, j, attn_ps):
                t = 2 * pair + j
                rhs = ma_ch[t // TPC][:, t % TPC, :]
                for nt in range(4):
                    nc.tensor.matmul(
                        attn_ps[nt][:],
                        pexp[:, j * 512 + nt * 128:j * 512 + (nt + 1) * 128],
                        rhs,
                        start=(t == 0), stop=(t == MT_TILES - 1),
                    )

            compute_loop = (tc.For_i(0, repeat, 1)
                            if repeat > 1 and loop_scope == "compute"
                            else contextlib.nullcontext())
            compute_loop.__enter__()
            for h in range(1 if "pass1" in ablate else 2):  # n-halves of 512
                n0 = h * 512
                attn_ps = [ps_a.tile([128, EMBED + 2], f32, tag="attn",
                                     name=f"attn_h{h}_{i}")
                           for i in range(4)]
                prev_exp = None

                # m-tiles processed in PAIRS: both tiles' scores land in one
                # 2-bank PSUM tile and ScalarE runs ONE exp per pair.  This
                # halves ACT per-instruction overhead and doubles the latency
                # budget for hiding exp behind the next pair's matmuls.
                for p in range(NPAIR):
                    scores = ps_s.tile([128, 1024], f32, tag="scores")
                    for j in range(2):
                        t = 2 * p + j
                        mt_k = mt_ch[t // TPC]
                        moff = (t % TPC) * 128
                        for e in range(2):
                            nc.tensor.matmul(
                                scores[:, j * 512:(j + 1) * 512],
                                mt_k[:, e, moff:moff + 128],
                                nt_sb[:, e, n0:n0 + 512],
                                start=(e == 0), stop=(e == 1),
                            )
                        # previous pair's mm2 interleaves between this pair's
                        # two mm1 blocks, so PE never waits on ScalarE
                        if prev_exp is not None:
                            _mm2(prev_exp[0], prev_exp[1], j, attn_ps)
                    expt = work.tile([128, 1024], bf16, tag="expt", bufs=3)
                    nc.scalar.activation(expt[:], scores[:], Exp,
                                         bias=eb[:], scale=1.0)
                    prev_exp = (p, expt)

                for j in range(2):
                    _mm2(prev_exp[0], prev_exp[1], j, attn_ps)

                if "noeplg" in ablate:
                    for nt in range(4):
                        nc.vector.tensor_copy(out_sb[:, 4 * h + nt, 0:EMBED],
                                              attn_ps[nt][:, 0:EMBED])
                        nc.sync.dma_start(out_ap[:, 4 * h + nt, :],
                                          out_sb[:, 4 * h + nt, :])
                    continue
                # ---- epilogue for this half ----
                # diag correction folded into the PSUM drain: U' = U - w * MD
                # (also corrects Z in col 256 and the gate dot in col 257).
                # The 2-col tails (Z, U.Wg) drain FIRST so the scalar gate
                # chain runs while the big 256-col drains proceed; drains and
                # blends are split DVE / Pool so the tail isn't one serial
                # engine queue.
                utail = work.tile([128, 4, 2], f32, tag="utail", bufs=2,
                                  name=f"utail_h{h}")
                for nt in range(4):
                    g = 4 * h + nt
                    nc.vector.scalar_tensor_tensor(
                        out=utail[:, nt, :],
                        in0=md_sb[:, g, EMBED:EMBED + 2],
                        scalar=negw_all[:, g:g + 1],
                        in1=attn_ps[nt][:, EMBED:EMBED + 2],
                        op0=mult, op1=add,
                    )
                # batched scalar chain over the 4 tiles: rz = 1/Z,
                # gd = (U.Wg) * rz, gate = 1/(1+exp(-(gd+b)))
                rz4 = work.tile([128, 4], f32, tag="rz4")
                nc.vector.reciprocal(rz4[:], utail[:, :, 0])
                gd4 = work.tile([128, 4], f32, tag="gd4")
                nc.vector.tensor_mul(gd4[:], utail[:, :, 1], rz4[:])
                # sigmoid via Exp so the ACT Exp table is never swapped:
                # gate = 1 / (1 + exp(-(gd + b)))
                ep4 = work.tile([128, 4], f32, tag="ep4")
                nc.scalar.activation(ep4[:], gd4[:], Exp,
                                     bias=gbn[:], scale=-1.0)
                ep14 = work.tile([128, 4], f32, tag="ep14")
                nc.vector.tensor_scalar_add(ep14[:], ep4[:], 1.0)
                gate4 = work.tile([128, 4], f32, tag="gate4")
                nc.vector.reciprocal(gate4[:], ep14[:])
                usb4 = work.tile([128, 4, EMBED], f32, tag="usb4", bufs=2,
                                 name=f"usb4_h{h}")
                for nt in range(4):
                    g = 4 * h + nt
                    nc.vector.scalar_tensor_tensor(
                        out=usb4[:, nt, :], in0=md_sb[:, g, 0:EMBED],
                        scalar=negw_all[:, g:g + 1],
                        in1=attn_ps[nt][:, 0:EMBED], op0=mult, op1=add,
                    )
                for nt in range(4):
                    g = 4 * h + nt
                    # dif = U*rz - N ; out = dif*gate + N
                    dif = work.tile([128, EMBED], f32, tag="dif", bufs=4)
                    nc.vector.scalar_tensor_tensor(
                        out=dif[:], in0=usb4[:, nt, :],
                        scalar=rz4[:, nt:nt + 1],
                        in1=nf_sb[:, g, :], op0=mult,
                        op1=mybir.AluOpType.subtract,
                    )
                    nc.vector.scalar_tensor_tensor(
                        out=out_sb[:, g, :], in0=dif[:],
                        scalar=gate4[:, nt:nt + 1],
                        in1=nf_sb[:, g, :], op0=mult, op1=add,
                    )
                    nc.sync.dma_start(out_ap[:, g, :], out_sb[:, g, :])
            compute_loop.__exit__(None, None, None)

    nc.compile()
    return nc


def _get_nc(repeat=1):
    key = f"nc{repeat}"
    if key not in _cache:
        _cache[key] = _build_nc(repeat)
    return _cache[key]


def build_in_maps(M, N, Wg, bg, gate_b):
    """Per-core input maps for the SPMD kernel (shared with test harness)."""
    import ml_dtypes

    M = np.ascontiguousarray(np.asarray(M, dtype=np.float32))
    N = np.ascontiguousarray(np.asarray(N, dtype=np.float32))
    Wg = np.asarray(Wg, dtype=np.float32).reshape(1, EMBED)
    bg = np.asarray(bg, dtype=np.float32).reshape(-1)
    gate_b = np.asarray(gate_b, dtype=np.float32).reshape(-1)

    MT = np.ascontiguousarray(M.T)
    ga = M @ Wg.reshape(EMBED, 1)  # [n, 1] gate-dot column
    MA = np.concatenate([M, np.ones((N_ROWS, 1), np.float32), ga], axis=1)
    # pre-transpose to [128, tiles, cols]: partition p holds rows b*128+p
    MAb = np.ascontiguousarray(
        MA.reshape(MT_TILES, 128, EMBED + 2).transpose(1, 0, 2)
        .astype(ml_dtypes.bfloat16))
    GB = np.full((128, 1), -(float(bg[0]) + float(gate_b[0])), np.float32)

    in_maps = []
    for c in range(NCORES):
        sl = slice(c * NLOC, (c + 1) * NLOC)
        Ns = N[sl]
        MDs = MA[sl]
        in_maps.append({
            "MT": MT,
            "MA": MAb,
            "NT": np.ascontiguousarray(Ns.T),
            "NF": np.ascontiguousarray(
                Ns.reshape(NT_TILES, 128, EMBED).transpose(1, 0, 2)),
            "MD": np.ascontiguousarray(
                MDs.reshape(NT_TILES, 128, EMBED + 2).transpose(1, 0, 2)),
            "GB": GB,
        })
    return in_maps


def _numpy_fallback(M, N, Wg, bg, gate_b, iseval):
    M64 = M.astype(np.float64)
    N64 = N.astype(np.float64)
    mt = N64 @ M64.T
    if not iseval:
        np.fill_diagonal(mt, 0.0)
    else:
        mt[0, :] = 0.0
    mt -= mt.max(axis=1, keepdims=True)
    e = np.exp(mt)
    attn = e / e.sum(axis=1, keepdims=True)
    out_attn = attn @ M64
    gate = 1.0 / (1.0 + np.exp(-(out_attn @ Wg.astype(np.float64).T
                                 + float(bg[0]) + float(gate_b[0]))))
    boosted = out_attn * gate + N64 * (1.0 - gate)
    return boosted[:, None, None, :].astype(np.float32)


def kernel(M, N, Wg, bg, gate_b, iseval):
    from concourse import bass_utils

    M = np.ascontiguousarray(np.asarray(M, dtype=np.float32))
    N = np.ascontiguousarray(np.asarray(N, dtype=np.float32))
    Wg = np.asarray(Wg, dtype=np.float32).reshape(1, EMBED)
    bg = np.asarray(bg, dtype=np.float32).reshape(-1)
    gate_b = np.asarray(gate_b, dtype=np.float32).reshape(-1)

    if int(np.asarray(iseval)) != 0:
        return _numpy_fallback(M, N, Wg, bg, gate_b, True)

    nc = _get_nc()
    in_maps = build_in_maps(M, N, Wg, bg, gate_b)

    res = bass_utils.run_bass_kernel_spmd(
        nc, in_maps, core_ids=list(range(NCORES)))
    # out comes back [128, NT_TILES, EMBED] per core: invert the transpose
    out = np.concatenate(
        [res.results[c]["out"].transpose(1, 0, 2).reshape(NLOC, EMBED)
         for c in range(NCORES)], axis=0)
    return out[:, None, None, :].astype(np.float32)


if __name__ == "__main__":
    rng = np.random.default_rng(0)
    M = rng.standard_normal((N_ROWS, EMBED)).astype(np.float32)
    N = rng.standard_normal((N_ROWS, EMBED)).astype(np.float32)
    Wg = (rng.standard_normal((1, EMBED)) * 0.06).astype(np.float32)
    bg = (rng.standard_normal((1,)) * 0.1).astype(np.float32)
    gb = (rng.standard_normal((1,)) * 0.1).astype(np.float32)
    out = kernel(M=M, N=N, Wg=Wg, bg=bg, gate_b=gb, iseval=0)
    ref = _numpy_fallback(M, N, Wg, bg, gb, False)
    d = out.astype(np.float64) - ref.astype(np.float64)
    fro = np.linalg.norm(d) / np.linalg.norm(ref.astype(np.float64))
    print("self-check max-elem rel:", np.abs(d).max() / np.abs(ref).max())
    print("self-check fro rel:", fro)
